# revision 10
# baseline (speedup 1.0000x reference)
"""ClusterisedLinearNetwork Trainium2 kernel — hybrid dense + cluster-sorted.

Math per token t (N=262144):
  enc[t] = NeRF positional encoding of X[t] (120 dims, 10 freqs x sin/cos x 6)
  rgb[t, j] = sum_k weights[k,t] * (W[3*cluster_ids[t,k]+j, :] . enc[t])

Two engine-complementary paths split the tokens per core:

DENSE path (N_D=6144 tokens/core): routing absorbed into a dense matmul via a
host-built weighted one-hot matrix Q [256, t]; 9 streamed PE cols/token
(6 contraction + 3 ones-reduce) but only ~1 Act col/token. PE-heavy.

SORTED path (26624 tokens/core): the (token,k)->cluster assignments are
sorted by cluster on the host and padded into G=512-column segments; per
segment one matmul  Wc.T[120,3].T @ enc_sorted[120,512]  with the tiny
stationary swap hidden behind the 512-col stream. 3 segments pack one
[67,512] PSUM tile at partition bases 0/32/64 (HW restriction). ~3.36 PE
cols/token but ~3.36 Act (sin) cols/token. Act-heavy.

Interleaving the two instruction streams balances PE ~= Act ~= DMA.

The host applies routing weights + k-sum to the sorted path's per-assignment
dot products (gather + multiply-add), transposes the dense path's output, and
zeroes masked rows — exactly the reference's einsum/where.
"""
import sys
sys.path.insert(0, '/opt/trn_rl_repo')
import numpy as np

N_TOK = 262144
N_CORES = 8
C = 256
F = 10
D = 120

# ---- dense path ----
N_D = 6144                       # dense tokens per core
T = 512                          # tokens per inner group (PSUM bank fp32)
TM = 2048                        # tokens per macro group (DMA/ACT batching)
NG = N_D // TM                   # dense macro groups per core (3)
GPM = TM // T                    # inner groups per macro (4)
NGRP_D = NG * GPM                # 12 dense inner groups

# ---- sorted path ----
N_B = N_TOK - N_CORES * N_D      # 212992 sorted-path tokens (global)
G = 512                          # columns per segment
NSEG = 168                       # segments per core (8*168=1344 >= E+8sigma)
NSEG_G = NSEG * N_CORES
SPG = 3                          # segments per PSUM tile / enc group
NGRP_B = NSEG // SPG             # 56 groups per core
COLS = NSEG * G                  # 86016 sorted columns per core
GCOLS = SPG * G                  # 1536 enc columns per group
OUTW = NGRP_B * G                # 28672 output cols per core ([9, OUTW])

_compiled = None


def _enc_tables():
    r = np.arange(D)
    f_arr = np.where(r < 60, r // 6, (r - 60) // 6)
    d_arr = np.where(r < 60, r % 6, (r - 60) % 6)
    phase = np.where(r < 60, 0.0, np.pi / 2)
    s_arr = (r >= 60).astype(np.int64)
    perm = f_arr * 12 + s_arr * 6 + d_arr
    return f_arr, d_arr, phase, perm


def _host_prep(X, W, weights, cluster_ids):
    import ml_dtypes
    bf = ml_dtypes.bfloat16
    X = np.asarray(X, dtype=np.float32)
    W = np.asarray(W, dtype=np.float32)
    weights = np.asarray(weights, dtype=np.float32)
    ids = np.asarray(cluster_ids).astype(np.int64)

    f_arr, d_arr, phase, perm = _enc_tables()

    # --- Y: range-reduced sin arguments, [120, N] int16 ---
    Xd = X[:, d_arr].astype(np.float64).T
    Y = Xd * (2.0 ** f_arr)[:, None] + phase[:, None]
    Y -= np.round(Y / (2 * np.pi)) * (2 * np.pi)
    Y = np.round(Y / np.pi * 32767.0).astype(np.int16)

    Wp = W[:, perm]                                        # [768, 120]
    mask = np.all(X[:, :3] == -1.0, axis=-1)

    # --- token split: per core, first N_D tokens dense, rest sorted ---
    NPC = N_TOK // N_CORES
    tok = np.arange(N_TOK)
    off = tok % NPC
    is_dense = off < N_D
    dense_tok = tok[is_dense]                              # ordered by (core, off)
    b_tok = tok[~is_dense]                                 # [N_B]

    # --- dense path inputs ---
    wq8 = np.empty((128, 6 * D), np.float32)
    for j in range(3):
        for h in range(2):
            blk = Wp[3 * (128 * h + np.arange(128)) + j, :]
            wq8[:, (j * 2 + h) * D:(j * 2 + h + 1) * D] = blk
    WQc = wq8.astype(bf)

    Qd = np.zeros((C, N_CORES * N_D), np.float32)          # dense-token columns
    dcol = np.arange(N_CORES * N_D)
    wd = weights[:, dense_tok]
    for k in range(3):
        np.add.at(Qd, (ids[dense_tok, k], dcol), wd[k])
    if mask[dense_tok].any():
        Qd[:, mask[dense_tok]] = 0.0
    Yd = Y[:, dense_tok]                                   # [120, 8*N_D]

    # --- sorted path: assignments of b_tok, sorted by cluster ---
    a_c = ids[b_tok].T.reshape(-1)                         # [3*N_B] k-major
    a_t = np.tile(b_tok, 3)
    order = np.argsort(a_c, kind='stable')
    sc = a_c[order]
    st = a_t[order]

    counts = np.bincount(a_c, minlength=C)
    nseg_c = -(-counts // G)
    nseg_used = int(nseg_c.sum())
    assert nseg_used <= NSEG_G, f"segment overflow: {nseg_used} > {NSEG_G}"
    seg_start_c = np.concatenate(([0], np.cumsum(nseg_c)))[:-1]
    run_start_c = np.concatenate(([0], np.cumsum(counts)))[:-1]
    P = seg_start_c[sc] * G + (np.arange(3 * N_B) - run_start_c[sc])

    seg_cluster = np.zeros(NSEG_G, np.int64)
    seg_cluster[:nseg_used] = np.repeat(np.arange(C), nseg_c)

    Ysrt = np.zeros((D, NSEG_G * G), np.int16)
    Ysrt[:, P] = Y[:, st]

    wsel_rows = (3 * seg_cluster[:, None] + np.arange(3)).reshape(-1)
    WSEL = np.ascontiguousarray(Wp[wsel_rows, :].T)
    WSEL[:, 3 * nseg_used:] = 0.0
    WSEL = WSEL.astype(bf)

    in_maps = []
    for c in range(N_CORES):
        dsl = slice(c * N_D, (c + 1) * N_D)
        Qc = Qd[:, dsl]
        in_maps.append({
            "Ysrt": np.ascontiguousarray(Ysrt[:, c * COLS:(c + 1) * COLS]),
            "WSEL": np.ascontiguousarray(WSEL[:, c * 3 * NSEG:(c + 1) * 3 * NSEG]),
            "Ycat": np.ascontiguousarray(Yd[:, dsl]),
            "Qcat": np.concatenate([Qc[0:128, :], Qc[128:256, :]],
                                   axis=1).astype(bf),
            "WQc": WQc,
        })

    inv = np.empty(3 * N_B, np.int64)
    inv[order] = np.arange(3 * N_B)
    Pk = P[inv].reshape(3, N_B)                            # [k, b-index] padded col
    ctx = {"Pk": Pk, "weights": weights, "mask": mask,
           "b_tok": b_tok, "dense_tok": dense_tok}
    return in_maps, ctx


def _combine(core_outs_b, core_outs_d, ctx):
    Rcat = np.stack([np.asarray(o, np.float32).reshape(-1) for o in core_outs_b])
    Pk, weights, mask = ctx["Pk"], ctx["weights"], ctx["mask"]
    b_tok, dense_tok = ctx["b_tok"], ctx["dense_tok"]
    out3 = np.zeros((N_TOK, 3), np.float32)
    # sorted path
    for k in range(3):
        pos = Pk[k]
        core = pos // COLS
        L = pos % COLS
        s, p = np.divmod(L, G)
        g, u = np.divmod(s, SPG)
        wk = weights[k][b_tok]
        for j in range(3):
            flat = (3 * u + j) * OUTW + g * G + p
            out3[b_tok, j] += wk * Rcat[core, flat]
    # dense path
    dsub = np.concatenate([np.asarray(o, np.float32) for o in core_outs_d],
                          axis=1)                          # [3, 8*N_D]
    out3[dense_tok] = dsub.T
    out3[mask] = 0.0
    return out3


def _build(reps=1):
    """Compile the per-core Bass kernel (SPMD; same program all 8 cores)."""
    global _compiled
    if _compiled is not None and _compiled[0] == reps:
        return _compiled[1]
    from concourse import bacc, tile, mybir
    from contextlib import ExitStack

    bf16 = mybir.dt.bfloat16
    f32 = mybir.dt.float32
    i16 = mybir.dt.int16

    nc = bacc.Bacc("TRN2", target_bir_lowering=False, debug=False,
                   num_devices=N_CORES)

    Ysrt = nc.dram_tensor("Ysrt", [D, COLS], i16, kind="ExternalInput")
    WSEL = nc.dram_tensor("WSEL", [D, 3 * NSEG], bf16, kind="ExternalInput")
    Ycat = nc.dram_tensor("Ycat", [D, N_D], i16, kind="ExternalInput")
    Qcat = nc.dram_tensor("Qcat", [128, 2 * N_D], bf16, kind="ExternalInput")
    WQc = nc.dram_tensor("WQc", [128, 6 * D], bf16, kind="ExternalInput")
    OUT = nc.dram_tensor("OUT", [9, OUTW], f32, kind="ExternalOutput")
    rgbh = nc.dram_tensor("rgbh", [3, N_D], f32, kind="ExternalOutput")

    SIN = mybir.ActivationFunctionType.Sin
    SSCL = float(np.pi / 32767.0)

    with tile.TileContext(nc) as tc:
        with tc.tile_pool(name="const", bufs=1) as cpool, \
             tc.tile_pool(name="by", bufs=3) as bypool, \
             tc.tile_pool(name="benc", bufs=3) as bepool, \
             tc.tile_pool(name="bstage", bufs=3) as bspool, \
             tc.tile_pool(name="dy", bufs=2) as dypool, \
             tc.tile_pool(name="denc", bufs=2) as depool, \
             tc.tile_pool(name="dq", bufs=2) as dqpool, \
             tc.tile_pool(name="dp", bufs=2) as dppool, \
             tc.tile_pool(name="drgb", bufs=2) as drpool, \
             tc.tile_pool(name="bps", bufs=3, space="PSUM") as bpspool, \
             tc.tile_pool(name="mall", bufs=1, space="PSUM") as mpool, \
             tc.tile_pool(name="rgbp", bufs=1, space="PSUM") as rppool:

            wsel = cpool.tile([D, 3 * NSEG], bf16)
            nc.sync.dma_start(wsel[:], WSEL.ap())
            wq = cpool.tile([128, 6 * D], bf16)
            nc.sync.dma_start(wq[:], WQc.ap())
            ones_t = nc.const_aps.tensor(1.0, (D, 1), bf16)

            rep_ctx = ExitStack()
            if reps > 1:
                rep_ctx.enter_context(tc.For_i(0, reps, 1))

            state = {}
            rgb66_by_macro = {}
            bstage_cur = {}

            def emit_b_group(g):
                y_sb = bypool.tile([D, GCOLS], i16, tag="by")
                e_sb = bepool.tile([D, GCOLS], bf16, tag="be")
                for h in range(2):
                    sl = slice(h * GCOLS // 2, (h + 1) * GCOLS // 2)
                    nc.sync.dma_start(
                        y_sb[:, sl],
                        Ysrt.ap()[:, g * GCOLS + h * GCOLS // 2:
                                  g * GCOLS + (h + 1) * GCOLS // 2])
                    nc.scalar.activation(e_sb[:, sl], y_sb[:, sl], SIN,
                                         bias=0.0, scale=SSCL)
                ps = bpspool.tile([67, G], f32, tag="bps")
                for u in range(SPG):
                    s = g * SPG + u
                    nc.tensor.matmul(
                        ps[32 * u:32 * u + 3, :],
                        lhsT=wsel[:, 3 * s:3 * s + 3],
                        rhs=e_sb[:, u * G:(u + 1) * G],
                        start=True, stop=True)
                gp, gh = divmod(g, 2)
                if gh == 0:
                    stage = bspool.tile([67, 2 * G], f32, tag="bst", name="bst")
                    bstage_cur[0] = stage
                stage = bstage_cur[0]
                nc.vector.tensor_copy(stage[:, gh * G:(gh + 1) * G], ps[:])
                if gh == 1:
                    for u in range(3):
                        nc.sync.dma_start(
                            OUT.ap()[3 * u:3 * u + 3,
                                     gp * 2 * G:(gp + 1) * 2 * G],
                            stage[32 * u:32 * u + 3, :])

            def emit_d_group(gg):
                if gg < NGRP_D:
                    m, g = divmod(gg, GPM)
                    moff = m * TM
                    goff = g * T
                    if g == 0:
                        y_sb = dypool.tile([D, TM], i16, tag="dy")
                        encT = depool.tile([D, TM], f32, tag="denc")
                        q_sb = dqpool.tile([128, 2, TM], bf16, tag="dq")
                        nsin = GPM if m == 0 else 1
                        for u in range(nsin):
                            usl = slice(u * TM // nsin, (u + 1) * TM // nsin)
                            nc.sync.dma_start(
                                y_sb[:, usl],
                                Ycat.ap()[:, moff + u * TM // nsin:
                                          moff + (u + 1) * TM // nsin])
                            nc.scalar.activation(encT[:, usl], y_sb[:, usl],
                                                 SIN, bias=0.0, scale=SSCL)
                        rgb66_by_macro[m] = drpool.tile([66, TM], f32,
                                                        tag="rgb66",
                                                        name="rgb66")
                        state["enc"], state["q"] = encT, q_sb
                        for h in range(2):
                            for u in range(2):
                                nc.sync.dma_start(
                                    q_sb[:, h, u * TM // 2:(u + 1) * TM // 2],
                                    Qcat.ap()[:, h * N_D + moff + u * TM // 2:
                                              h * N_D + moff + (u + 1) * TM // 2])
                    encT, q_sb = state["enc"], state["q"]
                    mall = mpool.tile([128, 3 * T], f32, tag="mall")
                    for j in range(3):
                        for h in range(2):
                            nc.tensor.matmul(
                                mall[0:D, j * T:(j + 1) * T],
                                lhsT=wq[:, (j * 2 + h) * D:(j * 2 + h + 1) * D],
                                rhs=q_sb[:, h, goff:goff + T],
                                start=(h == 0), stop=(h == 1))
                    p_all = dppool.tile([D, 3 * T], bf16, tag="dp")
                    nc.vector.tensor_tensor(
                        out=p_all[:].rearrange('p (j t) -> p j t', j=3),
                        in0=mall[0:D, :].rearrange('p (j t) -> p j t', j=3),
                        in1=encT[:, None, goff:goff + T].to_broadcast([D, 3, T]),
                        op=mybir.AluOpType.mult)
                    state[gg] = p_all
                pg = gg - 1
                if pg >= 0:
                    pm, pgr = divmod(pg, GPM)
                    pgoff = pgr * T
                    p_prev = state.pop(pg)
                    rgbp = rppool.tile([66, T], f32, tag="rgbp")
                    for j in range(3):
                        nc.tensor.matmul(
                            rgbp[32 * j:32 * j + 1, :],
                            lhsT=ones_t,
                            rhs=p_prev[:, j * T:(j + 1) * T],
                            start=True, stop=True)
                    rgb66p = rgb66_by_macro[pm]
                    nc.vector.tensor_copy(rgb66p[:, pgoff:pgoff + T], rgbp[:])
                    if pgr == GPM - 1:
                        nc.sync.dma_start(rgbh.ap()[:, pm * TM:(pm + 1) * TM],
                                          rgb66p[0:66:32, :])
                        del rgb66_by_macro[pm]

            # interleave: 13 dense steps (incl. flush), 56 B groups
            b_done = 0
            for gg in range(NGRP_D + 1):
                emit_d_group(gg)
                b_target = (gg + 1) * NGRP_B // (NGRP_D + 1)
                while b_done < b_target:
                    emit_b_group(b_done)
                    b_done += 1
            while b_done < NGRP_B:
                emit_b_group(b_done)
                b_done += 1
            rep_ctx.close()

    nc.compile()
    _compiled = (reps, nc)
    return nc


def kernel(X, W, weights, cluster_ids):
    from concourse import bass_utils

    nc = _build()
    in_maps, ctx = _host_prep(X, W, weights, cluster_ids)
    res = bass_utils.run_bass_kernel_spmd(nc, in_maps,
                                          core_ids=list(range(N_CORES)))
    return _combine([res.results[c]["OUT"] for c in range(N_CORES)],
                    [res.results[c]["rgbh"] for c in range(N_CORES)], ctx)


# revision 11
# speedup vs baseline: 2.4268x; 2.4268x over previous
"""ClusterisedLinearNetwork Trainium2 kernel.

Math: per token t (N=262144):
  enc[t] = NeRF positional encoding of X[t] (120 dims, 10 freqs x sin/cos x 6)
  out_all[t] = enc[t] @ W.T  -> [256 clusters, 3]
  rgb[t, j] = sum_k weights[k,t] * out_all[t, cluster_ids[t,k], j]

Device formulation (avoids materializing the 768-wide dense output):
  Q[c, t]  = sum_k weights[k,t] * [cluster_ids[t,k] == c]   (routing matrix, host-densified)
  M_j[d,t] = sum_c W[(c,j), d] * Q[c, t]                     (TensorE matmul)
  rgb[j,t] = sum_d enc[d,t] * M_j[d,t]                       (DVE mult + ones-matmul reduce)

Identical FLOP count to the dense matmul (768x120 per token), but the gather/
weighted-sum is absorbed into the contraction, so no 768-wide per-token
selection pass is needed.

Sharding: data-parallel over 8 NeuronCores along the token axis; W replicated.

Note from optimization session 2026-08-08: alternatives evaluated on HW —
cluster-sorted MoE segmentation (tiny [120,3] stationaries, 0.55-0.71 ns/col
with ~300-670 ns/matmul overhead from stationary swaps + 32-col PE tile mode)
and a dense+sorted hybrid (measured 209us vs 136us for this kernel with a
noise-robust large-reps protocol). This dense formulation runs within ~10% of
its PE streaming floor (294912 cols/core @ 2.4 GHz = 123us) and remained the
fastest correct design.
"""
import sys
sys.path.insert(0, '/opt/trn_rl_repo')
import numpy as np

N_TOK = 262144
N_CORES = 8
NPC = N_TOK // N_CORES          # 32768 tokens per core
C = 256                          # clusters
F = 10                           # freq bands
D = 120                          # encoding dim
T = 512                          # tokens per inner group (one PSUM bank fp32)
TM = 2048                        # tokens per macro group (DMA/ACT batching)
NG = NPC // TM                   # macro groups per core (16)
GPM = TM // T                    # inner groups per macro (4)

DT_Q = None                      # set in _build: mybir dtype for Q/WQ operands
USE_BF16 = True

_compiled = None


def _host_prep_shared(X, W, weights, cluster_ids):
    """Host-side input conditioning (layout + routing densification)."""
    X = np.asarray(X, dtype=np.float32)
    W = np.asarray(W, dtype=np.float32)
    weights = np.asarray(weights, dtype=np.float32)
    ids = np.asarray(cluster_ids).astype(np.int64)

    # --- row order for the encoding axis (d'): rows 0..59 sin(2^f x_d), 60..119 cos ---
    r = np.arange(D)
    f_arr = np.where(r < 60, r // 6, (r - 60) // 6)
    d_arr = np.where(r < 60, r % 6, (r - 60) % 6)
    phase = np.where(r < 60, 0.0, np.pi / 2)
    # original enc column for permuted row r: f*12 + s*6 + d
    s_arr = (r >= 60).astype(np.int64)
    perm = f_arr * 12 + s_arr * 6 + d_arr

    # --- Y: range-reduced sin arguments, [120, N] fp32 ---
    # xb = x_d * 2^f (+ pi/2 for cos rows); y = xb mod 2pi -> [-pi, pi]
    Xd = X[:, d_arr].astype(np.float64).T          # [120, N]
    Y = Xd * (2.0 ** f_arr)[:, None] + phase[:, None]
    Y -= np.round(Y / (2 * np.pi)) * (2 * np.pi)
    Y = np.round(Y / np.pi * 32767.0).astype(np.int16)

    # --- Q: weighted one-hot routing matrix [256, N] ---
    Q = np.zeros((C, N_TOK), np.float32)
    t_idx = np.arange(N_TOK)
    for k in range(3):
        np.add.at(Q, (ids[:, k], t_idx), weights[k, :])
    # masked tokens (first 3 coords all exactly -1) produce zero output
    mask = np.all(X[:, :3] == -1.0, axis=-1)
    if mask.any():
        Q[:, mask] = 0.0

    # --- WQ: lhsT blocks [c'=128, d=120] for (j, h) ---
    # WQcat[c', (j*2+h)*120 + d'] = W[3*(128*h + c') + j, perm[d']]
    Wp = W[:, perm]                                # [768, 120]
    WQcat = np.empty((128, 6 * D), np.float32)
    for j in range(3):
        for h in range(2):
            blk = Wp[3 * (128 * h + np.arange(128)) + j, :]   # [128, 120]
            WQcat[:, (j * 2 + h) * D:(j * 2 + h + 1) * D] = blk
    return Y, Q, WQcat


def _build(reps=1):
    """Compile the per-core Bass kernel (SPMD; same program all 8 cores)."""
    global _compiled
    if _compiled is not None and _compiled[0] == reps:
        return _compiled[1]
    from concourse import bacc, tile, mybir
    from contextlib import ExitStack

    dtq = mybir.dt.bfloat16 if USE_BF16 else mybir.dt.float32
    f32 = mybir.dt.float32

    nc = bacc.Bacc("TRN2", target_bir_lowering=False, debug=False,
                   num_devices=N_CORES)

    Ycat = nc.dram_tensor("Ycat", [D, NPC], mybir.dt.int16, kind="ExternalInput")
    Qcat = nc.dram_tensor("Qcat", [128, 2 * NPC], dtq, kind="ExternalInput")
    WQc = nc.dram_tensor("WQc", [128, 6 * D], dtq, kind="ExternalInput")
    rgbh = nc.dram_tensor("rgbh", [3, NPC], f32, kind="ExternalOutput")

    with tile.TileContext(nc) as tc:
        with tc.tile_pool(name="const", bufs=1) as cpool, \
             tc.tile_pool(name="ysl", bufs=3) as ypool, \
             tc.tile_pool(name="enc", bufs=3) as epool, \
             tc.tile_pool(name="q", bufs=3) as qpool, \
             tc.tile_pool(name="p", bufs=3) as ppool, \
             tc.tile_pool(name="rgb", bufs=2) as rpool, \
             tc.tile_pool(name="mall", bufs=2, space="PSUM") as mpool, \
             tc.tile_pool(name="rgbp", bufs=2, space="PSUM") as rppool:

            wq = cpool.tile([128, 6 * D], dtq)
            nc.sync.dma_start(wq[:], WQc.ap())
            bf16 = mybir.dt.bfloat16
            ones_t = nc.const_aps.tensor(1.0, (D, 1), bf16)

            rep_ctx = ExitStack()
            if reps > 1:
                rep_ctx.enter_context(tc.For_i(0, reps, 1))
            NGRP = NG * GPM
            state = {}          # per-group carried tiles for 1-group delay
            rgb66_by_macro = {}
            for gg in range(NGRP + 1):
                if gg < NGRP:
                    m, g = divmod(gg, GPM)
                    moff = m * TM
                    goff = g * T
                    if g == 0:
                        y_sb = ypool.tile([D, TM], mybir.dt.int16, tag="y")
                        encT = epool.tile([D, TM], f32, tag="enc")
                        q_sb = qpool.tile([128, 2, TM], dtq, tag="q")
                        nsin = GPM if m == 0 else 1
                        for u in range(nsin):
                            usl = slice(u * TM // nsin, (u + 1) * TM // nsin)
                            nc.sync.dma_start(y_sb[:, usl],
                                              Ycat.ap()[:, moff + u * TM // nsin:
                                                        moff + (u + 1) * TM // nsin])
                            nc.scalar.activation(encT[:, usl], y_sb[:, usl],
                                                 mybir.ActivationFunctionType.Sin,
                                                 bias=0.0,
                                                 scale=float(np.pi / 32767.0))
                        rgb66_by_macro[m] = rpool.tile([66, TM], f32, tag="rgb66", name="rgb66")
                        state["enc"], state["q"] = encT, q_sb
                    encT, q_sb = state["enc"], state["q"]
                    if g == 0:
                        for h in range(2):
                            for u in range(2):
                                nc.sync.dma_start(
                                    q_sb[:, h, u * TM // 2:(u + 1) * TM // 2],
                                    Qcat.ap()[:, h * NPC + moff + u * TM // 2:
                                              h * NPC + moff + (u + 1) * TM // 2])
                    mall = mpool.tile([128, 3 * T], f32, tag="mall")
                    for j in range(3):
                        for h in range(2):
                            nc.tensor.matmul(
                                mall[0:D, j * T:(j + 1) * T],
                                lhsT=wq[:, (j * 2 + h) * D:(j * 2 + h + 1) * D],
                                rhs=q_sb[:, h, goff:goff + T],
                                start=(h == 0), stop=(h == 1))
                    p_all = ppool.tile([D, 3 * T], bf16, tag="p")
                    nc.vector.tensor_tensor(
                        out=p_all[:].rearrange('p (j t) -> p j t', j=3),
                        in0=mall[0:D, :].rearrange('p (j t) -> p j t', j=3),
                        in1=encT[:, None, goff:goff + T].to_broadcast([D, 3, T]),
                        op=mybir.AluOpType.mult)
                    state[gg] = p_all
                # delayed-by-one reduce + evacuation
                pg = gg - 1
                if pg >= 0:
                    pm, pgr = divmod(pg, GPM)
                    pgoff = pgr * T
                    p_prev = state.pop(pg)
                    rgbp = rppool.tile([66, T], f32, tag="rgbp")
                    for j in range(3):
                        nc.tensor.matmul(
                            rgbp[32 * j:32 * j + 1, :],
                            lhsT=ones_t,
                            rhs=p_prev[:, j * T:(j + 1) * T],
                            start=True, stop=True)
                    rgb66p = rgb66_by_macro[pm]
                    nc.scalar.copy(rgb66p[:, pgoff:pgoff + T], rgbp[:])
                    if pgr == GPM - 1:
                        nc.sync.dma_start(rgbh.ap()[:, pm * TM:(pm + 1) * TM],
                                          rgb66p[0:66:32, :])
                        del rgb66_by_macro[pm]
            rep_ctx.close()

    nc.compile()
    _compiled = (reps, nc)
    return nc


def kernel(X, W, weights, cluster_ids, _want_trace=False, _trace_kwargs=None):
    from concourse import bass_utils
    import ml_dtypes

    nc = _build()
    Y, Q, WQcat = _host_prep_shared(X, W, weights, cluster_ids)

    np_q = ml_dtypes.bfloat16 if USE_BF16 else np.float32
    WQc_np = WQcat.astype(np_q)
    in_maps = []
    for c in range(N_CORES):
        sl = slice(c * NPC, (c + 1) * NPC)
        Qc = Q[:, sl]                              # [256, NPC]
        Qcat_np = np.concatenate([Qc[0:128, :], Qc[128:256, :]],
                                 axis=1).astype(np_q)   # [128, 2*NPC]
        in_maps.append({
            "Ycat": np.ascontiguousarray(Y[:, sl]),
            "Qcat": Qcat_np,
            "WQc": WQc_np,
        })

    kw = {}
    if _want_trace:
        kw = dict(trace=True, **(_trace_kwargs or {}))
    res = bass_utils.run_bass_kernel_spmd(nc, in_maps,
                                          core_ids=list(range(N_CORES)), **kw)
    out = np.empty((N_TOK, 3), np.float32)
    for c in range(N_CORES):
        out[c * NPC:(c + 1) * NPC, :] = np.asarray(res.results[c]["rgbh"]).T
    if _want_trace:
        return out, res
    return out
